# revision 14
# baseline (speedup 1.0000x reference)
"""Conv2DMod (StyleGAN-style modulated 3x3 conv) for 8 Trainium2 NeuronCores.

Math: out[b] = d[b,:] * conv2d(x[b], weight * (1+y[b])[None,:,None,None])
with d[b,o] = rsqrt(sum_{i,kh,kw} (weight[o,i,kh,kw]*(1+y[b,i]))^2 + eps).

Sharding: data-parallel over batch, one sample per core. Each core holds a
replica of the shared weight (fp16, lhsT layout), modulates it by its own
(1+y[b]) on-device, and runs the conv as 288 accumulating fp16 matmuls
(N=512 pixels, K=128 per tap).

The conv is ordered in four phases per oc half -- (ic0,kk0-4), (ic0,kk5-8),
(ic1,kk0-4), (ic1,kk5-8) -- with all 8 nk PSUM banks held open across the
phases. This lets the matmul stream start as soon as the first 5 ic0 weight
taps and the first x rows have streamed in (~0.5 MB), instead of waiting
for the full weight tensor.

The demodulation scale d (a per-sample [256] vector) is precomputed on the
host and shipped together with (1+y) as one tiny [128,4] f32 DMA; it is
applied when draining PSUM (f32) to fp16 output. Inputs stream in fp16
(half the HBM traffic of f32); output is fp16, upcast on host. DMA
descriptors are issued from both HWDGE queues (sync + scalar).
"""

import numpy as np

import concourse.bacc as bacc
import concourse.mybir as mybir
import concourse.tile as tile
from concourse.bass_utils import run_bass_kernel_spmd

B, C, H, W = 8, 256, 64, 64
O = 256
HP, WP = H + 2, W + 2  # 66x66 zero-padded image
EPS = 1e-6
F32 = mybir.dt.float32
F16 = mybir.dt.float16

WARM_N = 8  # dummy warm-up matmuls to ramp the PE clock while DMA streams

_CACHE = {}


def _build():
    nc = bacc.Bacc("TRN2", target_bir_lowering=False)
    xpad_d = nc.dram_tensor("xpad", [128, 2 * HP * WP], F16, kind="ExternalInput")
    wt_d = nc.dram_tensor("wt", [128, 18 * O], F16, kind="ExternalInput")
    ymd_d = nc.dram_tensor("ymd", [128, 4], F32, kind="ExternalInput")
    out_d = nc.dram_tensor("out", [2, 128, H * W], F16, kind="ExternalOutput")

    with tile.TileContext(nc) as tc:
        with (
            tc.tile_pool(name="big", bufs=1) as big,
            tc.tile_pool(name="small", bufs=1) as small,
            tc.tile_pool(name="outp", bufs=2) as outp,
            tc.tile_pool(name="cpsum", bufs=8, space="PSUM") as cpsum,
        ):
            w_all = big.tile([128, 18 * O], F16)
            x_all = big.tile([128, 2 * HP * WP], F16)
            ymd = small.tile([128, 4], F32)  # cols 0-1: (1+y_ic); 2-3: d[oc]
            warm_in = small.tile([128, 512], F16)
            nc.vector.memset(warm_in[:], 0.0)

            # --- DMA issue, split across both HWDGE queues -----------------
            def xdma(eng, ic, r0, r1):
                sl = slice(ic * HP * WP + r0 * WP, ic * HP * WP + r1 * WP)
                eng.dma_start(x_all[:, sl], xpad_d[:, sl])

            def wdma(eng, j0, j1):
                sl = slice(j0 * O, j1 * O)
                eng.dma_start(w_all[:, sl], wt_d[:, sl])

            # Order within a queue = priority; phase A needs only ymd,
            # w ic0 kk0-4, and the leading x ic0 rows. ic1 material is not
            # consumed until T+15.5us, so it rides at the back.
            nc.sync.dma_start(ymd[:], ymd_d[:])
            xdma(nc.scalar, 0, 0, 11)
            wdma(nc.sync, 0, 3)          # ic0 kk0-2
            wdma(nc.scalar, 3, 5)        # ic0 kk3-4
            wdma(nc.scalar, 5, 7)        # ic0 kk5-6
            wdma(nc.sync, 7, 9)          # ic0 kk7-8
            xdma(nc.scalar, 0, 11, 22)
            xdma(nc.sync, 0, 22, 33)
            xdma(nc.scalar, 0, 33, 44)
            xdma(nc.sync, 0, 44, 55)
            xdma(nc.scalar, 0, 55, 66)
            wdma(nc.scalar, 9, 18)       # all ic1 taps
            xdma(nc.sync, 1, 0, 22)
            xdma(nc.scalar, 1, 22, 44)
            xdma(nc.sync, 1, 44, 66)

            # modulate weights in place per DMA chunk: w[:, blk] *= (1+y_ic)
            for ic, k0, k1 in [(0, 0, 3), (0, 3, 5), (0, 5, 7), (0, 7, 9), (1, 0, 9)]:
                blk = w_all[:, (ic * 9 + k0) * O : (ic * 9 + k1) * O]
                nc.vector.tensor_scalar_mul(blk, blk, ymd[:, ic : ic + 1])

            # --- PE warm-up on zeros while input DMA streams ---------------
            warm_ps = cpsum.tile([128, 512], F32, tag="cps")
            for k in range(WARM_N):
                nc.tensor.matmul(
                    warm_ps[:], warm_in[:, 0:128], warm_in[:],
                    start=(k == 0), stop=(k == WARM_N - 1),
                )

            x_view = x_all.rearrange("p (c r q) -> p c r q", c=2, r=HP)
            PHASES = [(0, 0, 9), (1, 0, 9)]

            def drain(ps, oc, nk, osb, eng):
                nc.vector.tensor_scalar_mul(
                    osb[:, nk * 512 : (nk + 1) * 512], ps[:], ymd[:, 2 + oc : 3 + oc]
                )
                if nk % 2 == 1:  # DMA out two drained nk tiles at once
                    eng.dma_start(
                        out_d[oc, :, (nk - 1) * 512 : (nk + 1) * 512],
                        osb[:, (nk - 1) * 512 : (nk + 1) * 512],
                    )

            for oc in range(2):
                osb = outp.tile([128, H * W], F16, tag="osb")
                tiles = [
                    cpsum.tile([128, 512], F32, tag="cps", name=f"cps_{oc}_{i}")
                    for i in range(8)
                ]
                for pi, (ic, k0, k1) in enumerate(PHASES):
                    last_phase = pi == len(PHASES) - 1
                    for nk in range(8):
                        for kk in range(k0, k1):
                            kh, kw = divmod(kk, 3)
                            j = ic * 9 + kk
                            lhsT = w_all[:, j * O + oc * 128 : j * O + oc * 128 + 128]
                            rhs = x_view[
                                :, ic, nk * 8 + kh : nk * 8 + kh + 8, kw : kw + W
                            ]
                            nc.tensor.matmul(
                                tiles[nk], lhsT, rhs,
                                start=(pi == 0 and kk == k0),
                                stop=(last_phase and kk == k1 - 1),
                            )
                        if last_phase and not (oc == 1 and nk == 7):
                            eng = nc.sync if nk % 2 == 0 else nc.scalar
                            drain(tiles[nk], oc, nk, osb, eng)
                if oc == 1:
                    # last group: split the drain in two for a shorter tail;
                    # the first DMA also carries nk6 (drained, not yet sent)
                    ps_last = tiles[7]
                    lo, hi = 7 * 512, 7 * 512 + 256
                    nc.vector.tensor_scalar_mul(
                        osb[:, lo:hi], ps_last[:, 0:256], ymd[:, 3:4]
                    )
                    nc.sync.dma_start(out_d[1, :, 6 * 512 : hi], osb[:, 6 * 512 : hi])
                    nc.vector.tensor_scalar_mul(
                        osb[:, hi : hi + 256], ps_last[:, 256:512], ymd[:, 3:4]
                    )
                    nc.scalar.dma_start(
                        out_d[1, :, hi : hi + 256], osb[:, hi : hi + 256]
                    )
    nc.compile()
    return nc


def _get_nc():
    if "nc" not in _CACHE:
        _CACHE["nc"] = _build()
    return _CACHE["nc"]


def _prep_inputs(x, y, weight):
    x = np.ascontiguousarray(x, dtype=np.float32)
    y = np.ascontiguousarray(y, dtype=np.float32)
    weight = np.ascontiguousarray(weight, dtype=np.float32)
    # weight[o, i, kh, kw] -> wt[p, (ic*9+kk)*O + o] with i = ic*128+p
    wt = weight.transpose(2, 3, 1, 0).reshape(9, 2, 128, O).transpose(1, 0, 2, 3)
    wt = np.ascontiguousarray(wt.transpose(2, 0, 1, 3).reshape(128, 18 * O))
    wt16 = wt.astype(np.float16)
    # S[i, o] = sum_kk w[o, i, kk]^2 from the fp16 weights actually used
    w16f = wt16.astype(np.float64).reshape(128, 2, 9, O)
    S = (w16f**2).sum(axis=2)  # [128(p), 2(ic), O]
    in_maps = []
    for b in range(B):
        xp = np.pad(x[b], ((0, 0), (1, 1), (1, 1))).reshape(2, 128, HP * WP)
        xp = np.ascontiguousarray(
            xp.transpose(1, 0, 2).reshape(128, 2 * HP * WP).astype(np.float16)
        )
        ym1 = 1.0 + y[b].reshape(2, 128).T.astype(np.float64)  # [128, 2]
        # d[o] = rsqrt(sum_i (1+y_i)^2 S[i, o] + eps), o = oc*128 + p
        dd = 1.0 / np.sqrt(np.einsum("pc,pco->o", ym1**2, S) + EPS)
        ymd = np.empty((128, 4), np.float32)
        ymd[:, 0:2] = ym1
        ymd[:, 2:4] = dd.reshape(2, 128).T
        in_maps.append({"xpad": xp, "wt": wt16, "ymd": ymd})
    return in_maps


def kernel(x, y, weight, _run_kwargs=None):
    nc = _get_nc()
    in_maps = _prep_inputs(x, y, weight)
    kwargs = _run_kwargs or {}
    res = run_bass_kernel_spmd(nc, in_maps, core_ids=list(range(B)), **kwargs)
    out = np.empty((B, O, H, W), dtype=np.float32)
    for b in range(B):
        out[b] = res.results[b]["out"].astype(np.float32).reshape(O, H, W)
    if _run_kwargs is not None:
        _CACHE["last_result"] = res
    return out
